# revision 1
# baseline (speedup 1.0000x reference)
"""Trainium2 Bass kernel for nn_KnowledgeBaseLookup.

Computation (see reference):
    lookup = knowledge_base[indexes]            # (B,T,K,D) gather
    y      = einsum('btk,btkd->btd', weights, lookup)
    out    = y @ w_out.T + b_out                # (B,T,E)

Sharding: data-parallel over the B*T token dim across 8 cores; the
knowledge_base table is replicated per core.

Per-core layout (core owns 1024 tokens = 16384 gathered rows):
  - tokens are processed in 8 groups of 128 tokens (2048 rows).
  - one indirect DMA per group gathers 2048 rows of the table into a
    [128, 16*256] SBUF tile: slab s, partition p holds row (s*128+p).
  - stage 1 (weighted sum, transposed output): for each slab s and each
    128-wide d-chunk c, matmul(lhsT=gathered[:, s,c-chunk] [128,128],
    rhs=weight-mask [128,8]) -> yT_psum[c][:, s*8:(s+1)*8].  The weight
    mask column j carries weights[row] iff row's token == s*8+j, so the
    PE does the "multiply by weight + reduce over K=16" in one pass and
    produces y TRANSPOSED (d on partitions) - which stage 2 needs.
  - stage 2 (out_proj): matmul(lhsT=yT[:, c] [128d,128tok],
    rhs=w_out.T chunk [128d, 512e]) accumulated over the 2 d-chunks
    -> out_psum [128 tok, 512 e]; add bias; DMA out.
"""

import numpy as np

B, T, K = 4, 2048, 16
C, D, E = 262144, 256, 512
NCORES = 8
NTOK = B * T                      # 8192 tokens
TPC = NTOK // NCORES              # 1024 tokens per core
P = 128
GROUPS = TPC // P                 # 8 groups of 128 tokens per core
SLABS = (P * K) // P              # 16 slabs of 128 rows per group

_CACHE = {}


def _build_bass(repeats=1):
    import concourse.bass as bass
    import concourse.mybir as mybir
    from concourse import bacc
    from concourse.tile import TileContext

    fp32 = mybir.dt.float32
    nc = bacc.Bacc(
        "TRN2", target_bir_lowering=False, debug=False, num_devices=NCORES
    )

    kb = nc.dram_tensor("kb", [C, D], fp32, kind="ExternalInput")
    idx = nc.dram_tensor("idx", [P, GROUPS * SLABS], mybir.dt.int32,
                         kind="ExternalInput")
    wmask = nc.dram_tensor("wmask", [P, GROUPS * P], fp32, kind="ExternalInput")
    wout = nc.dram_tensor("wout", [P, 2 * E], fp32, kind="ExternalInput")
    bias = nc.dram_tensor("bias", [P, E], fp32, kind="ExternalInput")
    out = nc.dram_tensor("out", [TPC, E], fp32, kind="ExternalOutput")

    with TileContext(nc) as tc:
        with (
            tc.tile_pool(name="const", bufs=1) as cpool,
            tc.tile_pool(name="gather", bufs=2) as gpool,
            tc.tile_pool(name="y", bufs=2) as ypool,
            tc.tile_pool(name="osb", bufs=2) as opool,
            tc.tile_pool(name="psy", bufs=2, space="PSUM") as psy,
            tc.tile_pool(name="pso", bufs=2, space="PSUM") as pso,
        ):
            idx_sb = cpool.tile([P, GROUPS * SLABS], mybir.dt.int32)
            nc.sync.dma_start(out=idx_sb[:], in_=idx[:, :])
            wm_sb = cpool.tile([P, GROUPS * P], fp32)
            nc.sync.dma_start(out=wm_sb[:], in_=wmask[:, :])
            wo_sb = cpool.tile([P, 2 * E], fp32)
            nc.sync.dma_start(out=wo_sb[:], in_=wout[:, :])
            b_sb = cpool.tile([P, E], fp32)
            nc.sync.dma_start(out=b_sb[:], in_=bias[:, :])

            for g in range(GROUPS * repeats):
                g = g % GROUPS
                gath = gpool.tile([P, SLABS * D], fp32, tag="gath")
                # HW indirect DMA consumes ONE index per partition: gather
                # each 128-row slab with its own call (offset AP [128, 1],
                # dest [128, 256]).
                for s in range(SLABS):
                    col = g * SLABS + s
                    nc.gpsimd.indirect_dma_start(
                        out=gath[:, s * D:(s + 1) * D],
                        out_offset=None,
                        in_=kb[:, :],
                        in_offset=bass.IndirectOffsetOnAxis(
                            ap=idx_sb[:, col:col + 1], axis=0
                        ),
                    )

                yt0 = psy.tile([P, P], fp32, tag="yt0")
                yt1 = psy.tile([P, P], fp32, tag="yt1")
                for s in range(SLABS):
                    for c2, yt in enumerate((yt0, yt1)):
                        nc.tensor.matmul(
                            out=yt[:, s * 8:(s + 1) * 8],
                            lhsT=gath[:, s * D + c2 * P: s * D + (c2 + 1) * P],
                            rhs=wm_sb[:, g * P + s * 8: g * P + (s + 1) * 8],
                            start=True,
                            stop=True,
                        )

                y_sb = ypool.tile([P, D], fp32, tag="y")
                nc.vector.tensor_copy(out=y_sb[:, 0:P], in_=yt0[:])
                nc.vector.tensor_copy(out=y_sb[:, P:D], in_=yt1[:])

                o_ps = pso.tile([P, E], fp32, tag="ops")
                for c2 in range(2):
                    nc.tensor.matmul(
                        out=o_ps[:],
                        lhsT=y_sb[:, c2 * P:(c2 + 1) * P],
                        rhs=wo_sb[:, c2 * E:(c2 + 1) * E],
                        start=(c2 == 0),
                        stop=(c2 == 1),
                    )

                o_sb = opool.tile([P, E], fp32, tag="o")
                nc.vector.tensor_add(out=o_sb[:], in0=o_ps[:], in1=b_sb[:])
                nc.sync.dma_start(out=out[g * P:(g + 1) * P, :], in_=o_sb[:])

    nc.compile()
    return nc


def _host_prep(weights, indexes, w_out, b_out):
    """Build per-core input maps (everything except the replicated table)."""
    wflat = np.ascontiguousarray(weights, dtype=np.float32).reshape(NTOK, K)
    iflat = np.ascontiguousarray(indexes).reshape(NTOK, K).astype(np.int32)

    # w_out.T split into two 128-row d-chunks, chunk-major along free dim
    woutT = np.ascontiguousarray(w_out, dtype=np.float32).T  # [D, E]
    wout_host = np.ascontiguousarray(
        woutT.reshape(2, P, E).transpose(1, 0, 2).reshape(P, 2 * E)
    )
    bias_host = np.ascontiguousarray(
        np.broadcast_to(np.asarray(b_out, dtype=np.float32), (P, E))
    )
    # mask[p, j] = 1 iff partition p belongs to slab-local token j
    mask = (np.arange(P)[:, None] // K == np.arange(8)[None, :]).astype(np.float32)

    in_maps = []
    for c in range(NCORES):
        ic = iflat[c * TPC:(c + 1) * TPC].reshape(GROUPS, SLABS, P)
        wc = wflat[c * TPC:(c + 1) * TPC].reshape(GROUPS, SLABS, P)
        idx_host = np.ascontiguousarray(
            ic.transpose(2, 0, 1).reshape(P, GROUPS * SLABS)
        )
        w4 = wc[:, :, :, None] * mask[None, None, :, :]      # [G, S, P, 8]
        wmask_host = np.ascontiguousarray(
            w4.transpose(2, 0, 1, 3).reshape(P, GROUPS * P)
        )
        in_maps.append({
            "idx": idx_host,
            "wmask": wmask_host,
            "wout": wout_host,
            "bias": bias_host,
        })
    return in_maps


def kernel(weights, indexes, knowledge_base, w_out, b_out):
    from concourse.bass_utils import run_bass_kernel_spmd

    if "nc" not in _CACHE:
        _CACHE["nc"] = _build_bass()
    nc = _CACHE["nc"]

    kb_host = np.ascontiguousarray(knowledge_base, dtype=np.float32)
    in_maps = _host_prep(weights, indexes, w_out, b_out)
    for m in in_maps:
        m["kb"] = kb_host

    res = run_bass_kernel_spmd(nc, in_maps, list(range(NCORES)))
    out = np.concatenate([res.results[c]["out"] for c in range(NCORES)], axis=0)
    return out.reshape(B, T, E)



# revision 3
# speedup vs baseline: 1.7525x; 1.7525x over previous
"""Trainium2 Bass kernel for nn_KnowledgeBaseLookup (bucketed dma_gather design).

Computation (see reference):
    lookup = knowledge_base[indexes]            # (B,T,K,D) gather
    y      = einsum('btk,btkd->btd', weights, lookup)
    out    = y @ w_out.T + b_out                # (B,T,E)

Sharding: data-parallel over the B*T token dim across 8 cores; the
knowledge_base table is replicated per core.

Per-core design (1024 tokens, 16384 gathered rows):
  The old per-slab indirect-DMA gather paid a ~1us SWDGE desc-gen fixed cost
  per 128 rows (128 Pool instructions -> Pool-bound at ~140us).  Instead we
  use the batched `dma_gather` custom op (one instruction per 1024 rows), at
  the price of int16 indices: indices are bucketed by table chunk of 32768
  rows so chunk-local indices fit in int16, with the chunk base carried by
  the in_ap view.

  Layout: tokens split into 2 halves of 512; each half into 8 subgroups of
  64 tokens.  For each (half h, chunk b) one dma_gather call fetches 1024
  rows = 8 slabs of 128 slots; slab j holds up to 128 (token,k) pairs of
  subgroup j whose table row lies in chunk b (capacity = the mean occupancy,
  128).  Overflow pairs go to a per-half spill region of 4 slabs gathered by
  classic indirect DMA (any chunk, int32 indices).

  Reduction: for each slab, a [128,64] fp32r mask M[slot, j] =
  w[slot] * (tokloc[slot] == j) is built on DVE (is_equal on an iota table,
  then multiply; tokloc/weights are host-prepped per slot).  PE matmuls
  lhsT=rows (fp32r, a free bitcast of the gathered fp32) x rhs=mask
  accumulate yT[d, token] into PSUM; the spill slabs use a 512-wide mask
  over the whole half and accumulate last.  Stage 2 (out_proj) contracts
  yT with w_out.T (fp32r) per 128-token group, adds bias on DVE, DMAs out.

  The dma_gather Q7 ucode reads index i of a call from the idx tile at
  [16 + i%16, i//16] on the NEFF path (queue 0 channel base), while the
  bass-level interpreter reads [i%16, i//16]; the host writes both bands.
"""

import numpy as np

B, T, K = 4, 2048, 16
C, D, E = 262144, 256, 512
NCORES = 8
NTOK = B * T                      # 8192 tokens
TPC = NTOK // NCORES              # 1024 tokens per core
P = 128
HALVES = 2
HTOK = TPC // HALVES              # 512 tokens per half
NB = 8                            # value chunks
CHUNK = C // NB                   # 32768 rows, int16-addressable
NW = 8                            # subgroups per half
WTOK = HTOK // NW                 # 64 tokens per subgroup
NIDX_CALL = NW * P                # 1024 indices per dma_gather call
SPILL_SLABS = 4                   # per half
SPILL_CAP = SPILL_SLABS * P       # 512
MAIN_SLABS = HALVES * NB * NW     # 128
SPILL_TOT = HALVES * SPILL_SLABS  # 8

_CACHE = {}


def _build_bass():
    import concourse.bass as bass
    import concourse.mybir as mybir
    from concourse import bacc, library_config
    from concourse.tile import TileContext

    fp32 = mybir.dt.float32
    f32r = mybir.dt.float32r
    i16 = mybir.dt.int16
    i32 = mybir.dt.int32
    eq = mybir.AluOpType.is_equal
    mul = mybir.AluOpType.mult
    nc = bacc.Bacc(
        "TRN2", target_bir_lowering=False, debug=False, num_devices=NCORES
    )

    kb = nc.dram_tensor("kb", [C, D], f32r, kind="ExternalInput")
    idx16 = nc.dram_tensor("idx16", [P, HALVES * NB * (NIDX_CALL // 16)], i16,
                           kind="ExternalInput")
    idxsp = nc.dram_tensor("idxsp", [P, SPILL_TOT], i32, kind="ExternalInput")
    wslot = nc.dram_tensor("wslot", [P, MAIN_SLABS], fp32, kind="ExternalInput")
    tokloc = nc.dram_tensor("tokloc", [P, MAIN_SLABS], i16, kind="ExternalInput")
    wsp = nc.dram_tensor("wsp", [P, SPILL_TOT], fp32, kind="ExternalInput")
    toksp = nc.dram_tensor("toksp", [P, SPILL_TOT], i16, kind="ExternalInput")
    iota64 = nc.dram_tensor("iota64", [P, WTOK], i16, kind="ExternalInput")
    iota512 = nc.dram_tensor("iota512", [P, HTOK], i16, kind="ExternalInput")
    wout = nc.dram_tensor("wout", [P, 2 * E], f32r, kind="ExternalInput")
    bias = nc.dram_tensor("bias", [P, E], fp32, kind="ExternalInput")
    out = nc.dram_tensor("out", [TPC, E], fp32, kind="ExternalOutput")

    COLS = NIDX_CALL // 16  # idx16 columns per call

    with TileContext(nc) as tc:
        with (
            tc.tile_pool(name="const", bufs=1) as cpool,
            tc.tile_pool(name="gath", bufs=10) as gpool,
            tc.tile_pool(name="mask", bufs=3) as mpool,
            tc.tile_pool(name="spill", bufs=2) as sppool,
            tc.tile_pool(name="spmask", bufs=2) as smpool,
            tc.tile_pool(name="y", bufs=2) as ypool,
            tc.tile_pool(name="o", bufs=2) as opool,
            tc.tile_pool(name="psy", bufs=2, space="PSUM") as psy,
            tc.tile_pool(name="pso", bufs=2, space="PSUM") as pso,
        ):
            nc.gpsimd.load_library(library_config.mlp)

            idx_sb = cpool.tile([P, HALVES * NB * COLS], i16)
            nc.sync.dma_start(out=idx_sb[:], in_=idx16[:, :])
            idxsp_sb = cpool.tile([P, SPILL_TOT], i32)
            nc.sync.dma_start(out=idxsp_sb[:], in_=idxsp[:, :])
            w_sb = cpool.tile([P, MAIN_SLABS], fp32)
            nc.sync.dma_start(out=w_sb[:], in_=wslot[:, :])
            tl_sb = cpool.tile([P, MAIN_SLABS], i16)
            nc.sync.dma_start(out=tl_sb[:], in_=tokloc[:, :])
            wsp_sb = cpool.tile([P, SPILL_TOT], fp32)
            nc.sync.dma_start(out=wsp_sb[:], in_=wsp[:, :])
            tsp_sb = cpool.tile([P, SPILL_TOT], i16)
            nc.sync.dma_start(out=tsp_sb[:], in_=toksp[:, :])
            io64_sb = cpool.tile([P, WTOK], i16)
            nc.sync.dma_start(out=io64_sb[:], in_=iota64[:, :])
            io512_sb = cpool.tile([P, HTOK], i16)
            nc.sync.dma_start(out=io512_sb[:], in_=iota512[:, :])
            wo_sb = cpool.tile([P, 2 * E], f32r)
            nc.sync.dma_start(out=wo_sb[:], in_=wout[:, :])
            b_sb = cpool.tile([P, E], fp32)
            nc.sync.dma_start(out=b_sb[:], in_=bias[:, :])

            for h in range(HALVES):
                yt = psy.tile([P, 2 * HTOK], fp32, tag="yt")
                # start=True zeroes the whole 2KB psum zero-region, which
                # would wipe earlier 64-col writes in the same bank: zero the
                # banks once and accumulate-only (start=False everywhere).
                nc.vector.memset(yt[:], 0.0)

                gs = []
                for b in range(NB):
                    g = gpool.tile([P, NW, D], f32r, tag="g")
                    col0 = (h * NB + b) * COLS
                    nc.gpsimd.dma_gather(
                        out_ap=g[:],
                        in_ap=kb[b * CHUNK:(b + 1) * CHUNK, :],
                        idxs_ap=idx_sb[:, col0:col0 + COLS],
                        num_idxs=NIDX_CALL,
                        num_idxs_reg=NIDX_CALL,
                        elem_size=D,
                    )
                    gs.append(g)

                sp = sppool.tile([P, SPILL_SLABS, D], f32r, tag="sp")
                for s in range(SPILL_SLABS):
                    col = h * SPILL_SLABS + s
                    nc.gpsimd.indirect_dma_start(
                        out=sp[:, s, :],
                        out_offset=None,
                        in_=kb[:, :],
                        in_offset=bass.IndirectOffsetOnAxis(
                            ap=idxsp_sb[:, col:col + 1], axis=0
                        ),
                    )

                # mask-matmul reduction, bucket by bucket
                for b in range(NB):
                    blk = (h * NB + b) * NW
                    mask = mpool.tile([P, NW, WTOK], f32r, tag="m")
                    nc.vector.tensor_tensor(
                        out=mask[:],
                        in0=io64_sb[:].unsqueeze(1).broadcast_to([P, NW, WTOK]),
                        in1=tl_sb[:, blk:blk + NW].unsqueeze(2)
                            .broadcast_to([P, NW, WTOK]),
                        op=eq,
                    )
                    nc.vector.tensor_tensor(
                        out=mask[:],
                        in0=mask[:],
                        in1=w_sb[:, blk:blk + NW].unsqueeze(2)
                            .broadcast_to([P, NW, WTOK]),
                        op=mul,
                    )
                    for j in range(NW):
                        for ch in range(2):
                            nc.tensor.matmul(
                                out=yt[:, ch * HTOK + j * WTOK:
                                       ch * HTOK + (j + 1) * WTOK],
                                lhsT=gs[b][:, j, ch * P:(ch + 1) * P],
                                rhs=mask[:, j, :],
                                start=False,
                                stop=False,
                                skip_group_check=True,
                            )

                msp = smpool.tile([P, SPILL_SLABS, HTOK], f32r, tag="msp")
                sblk = h * SPILL_SLABS
                nc.vector.tensor_tensor(
                    out=msp[:],
                    in0=io512_sb[:].unsqueeze(1)
                        .broadcast_to([P, SPILL_SLABS, HTOK]),
                    in1=tsp_sb[:, sblk:sblk + SPILL_SLABS].unsqueeze(2)
                        .broadcast_to([P, SPILL_SLABS, HTOK]),
                    op=eq,
                )
                nc.vector.tensor_tensor(
                    out=msp[:],
                    in0=msp[:],
                    in1=wsp_sb[:, sblk:sblk + SPILL_SLABS].unsqueeze(2)
                        .broadcast_to([P, SPILL_SLABS, HTOK]),
                    op=mul,
                )
                for s in range(SPILL_SLABS):
                    for ch in range(2):
                        nc.tensor.matmul(
                            out=yt[:, ch * HTOK:(ch + 1) * HTOK],
                            lhsT=sp[:, s, ch * P:(ch + 1) * P],
                            rhs=msp[:, s, :],
                            start=False,
                            stop=(s == SPILL_SLABS - 1),
                            skip_group_check=True,
                        )

                yb = ypool.tile([P, 2 * HTOK], f32r, tag="yb")
                nc.vector.tensor_copy(out=yb[:], in_=yt[:])

                for g4 in range(HTOK // P):
                    ops = pso.tile([P, E], fp32, tag="ops")
                    for ch in range(2):
                        nc.tensor.matmul(
                            out=ops[:],
                            lhsT=yb[:, ch * HTOK + g4 * P:
                                    ch * HTOK + (g4 + 1) * P],
                            rhs=wo_sb[:, ch * E:(ch + 1) * E],
                            start=(ch == 0),
                            stop=(ch == 1),
                        )
                    osb = opool.tile([P, E], fp32, tag="osb")
                    nc.vector.tensor_add(out=osb[:], in0=ops[:], in1=b_sb[:])
                    row0 = (h * (HTOK // P) + g4) * P
                    nc.sync.dma_start(out=out[row0:row0 + P, :], in_=osb[:])

    nc.compile()
    return nc


def _host_prep(weights, indexes, w_out, b_out):
    """Bucket/sort (token,k) pairs per core and build all device-side arrays."""
    wflat = np.ascontiguousarray(weights, dtype=np.float32).reshape(NTOK, K)
    iflat = np.ascontiguousarray(indexes).reshape(NTOK, K).astype(np.int64)

    woutT = np.ascontiguousarray(w_out, dtype=np.float32).T      # [D, E]
    wout_host = np.ascontiguousarray(
        woutT.reshape(2, P, E).transpose(1, 0, 2).reshape(P, 2 * E)
    )
    bias_host = np.ascontiguousarray(
        np.broadcast_to(np.asarray(b_out, dtype=np.float32), (P, E))
    )
    iota64_h = np.ascontiguousarray(
        np.broadcast_to(np.arange(WTOK, dtype=np.int16), (P, WTOK))
    )
    iota512_h = np.ascontiguousarray(
        np.broadcast_to(np.arange(HTOK, dtype=np.int16), (P, HTOK))
    )

    COLS = NIDX_CALL // 16
    in_maps = []
    for c in range(NCORES):
        ic = iflat[c * TPC:(c + 1) * TPC].ravel()          # [16384]
        wc = wflat[c * TPC:(c + 1) * TPC].ravel()
        t = np.repeat(np.arange(TPC, dtype=np.int64), K)   # token per pair

        h = t // HTOK
        wsub = (t % HTOK) // WTOK
        b = ic // CHUNK
        key = (h * NB + b) * NW + wsub                     # 0..127 slab id

        order = np.argsort(key, kind="stable")
        ks = key[order]
        iv = ic[order]
        wv = wc[order]
        tv = t[order]
        starts = np.searchsorted(ks, np.arange(MAIN_SLABS))
        rank = np.arange(TPC * K) - starts[ks]

        idx16_host = np.zeros((P, HALVES * NB * COLS), np.int16)
        wslot_host = np.zeros((P, MAIN_SLABS), np.float32)
        tokloc_host = np.zeros((P, MAIN_SLABS), np.int16)
        idxsp_host = np.zeros((P, SPILL_TOT), np.int32)
        wsp_host = np.zeros((P, SPILL_TOT), np.float32)
        toksp_host = np.zeros((P, SPILL_TOT), np.int16)

        main = rank < P
        mk, mr = ks[main], rank[main]
        mi, mw, mt = iv[main], wv[main], tv[main]
        mh = mk // (NB * NW)
        mb = (mk // NW) % NB
        mj = mk % NW
        slot = mj * P + mr                                 # slot within call
        col = (mh * NB + mb) * COLS + slot // 16
        idx_local = (mi - mb * CHUNK).astype(np.int16)
        idx16_host[slot % 16, col] = idx_local             # interp layout
        idx16_host[16 + slot % 16, col] = idx_local        # NEFF Q7 layout
        wslot_host[mr, mk] = mw
        tokloc_host[mr, mk] = (mt - (mh * HTOK + mj * WTOK)).astype(np.int16)

        sh = ks[~main] // (NB * NW)                        # spill half
        si, sw, st = iv[~main], wv[~main], tv[~main]
        for hh in range(HALVES):
            sel = sh == hh
            n = int(sel.sum())
            if n > SPILL_CAP:
                raise ValueError(
                    f"spill overflow: core {c} half {hh} needs {n} > {SPILL_CAP}"
                )
            r = np.arange(n)
            idxsp_host[r % P, hh * SPILL_SLABS + r // P] = si[sel]
            wsp_host[r % P, hh * SPILL_SLABS + r // P] = sw[sel]
            toksp_host[r % P, hh * SPILL_SLABS + r // P] = (
                st[sel] - hh * HTOK
            ).astype(np.int16)

        in_maps.append({
            "idx16": idx16_host,
            "idxsp": idxsp_host,
            "wslot": wslot_host,
            "tokloc": tokloc_host,
            "wsp": wsp_host,
            "toksp": toksp_host,
            "iota64": iota64_h,
            "iota512": iota512_h,
            "wout": wout_host,
            "bias": bias_host,
        })
    return in_maps


def kernel(weights, indexes, knowledge_base, w_out, b_out):
    from concourse.bass_utils import run_bass_kernel_spmd

    if "nc" not in _CACHE:
        _CACHE["nc"] = _build_bass()
    nc = _CACHE["nc"]

    kb_host = np.ascontiguousarray(knowledge_base, dtype=np.float32)
    in_maps = _host_prep(weights, indexes, w_out, b_out)
    for m in in_maps:
        m["kb"] = kb_host

    res = run_bass_kernel_spmd(nc, in_maps, list(range(NCORES)))
    out = np.concatenate([res.results[c]["out"] for c in range(NCORES)], axis=0)
    return out.reshape(B, T, E).astype(np.float32)


# revision 4
# speedup vs baseline: 1.9845x; 1.1324x over previous
"""Trainium2 Bass kernel for nn_KnowledgeBaseLookup (bucketed dma_gather design).

Computation (see reference):
    lookup = knowledge_base[indexes]            # (B,T,K,D) gather
    y      = einsum('btk,btkd->btd', weights, lookup)
    out    = y @ w_out.T + b_out                # (B,T,E)

Sharding: data-parallel over the B*T token dim across 8 cores; the
knowledge_base table is replicated per core.

Per-core design (1024 tokens, 16384 gathered rows):
  The old per-slab indirect-DMA gather paid a ~1us SWDGE desc-gen fixed cost
  per 128 rows (128 Pool instructions -> Pool-bound at ~140us).  Instead we
  use the batched `dma_gather` custom op (one instruction per 1024 rows), at
  the price of int16 indices: indices are bucketed by table chunk of 32768
  rows so chunk-local indices fit in int16, with the chunk base carried by
  the in_ap view.

  Layout: tokens split into 2 halves of 512; each half into 8 subgroups of
  64 tokens.  For each (half h, chunk b) one dma_gather call fetches 1024
  rows = 8 slabs of 128 slots; slab j holds up to 128 (token,k) pairs of
  subgroup j whose table row lies in chunk b (capacity = the mean occupancy,
  128).  Overflow pairs go to a per-half spill region of 4 slabs gathered by
  classic indirect DMA (any chunk, int32 indices).

  Reduction: for each slab, a [128,64] fp32r mask M[slot, j] =
  w[slot] * (tokloc[slot] == j) is built on DVE (is_equal on an iota table,
  then multiply; tokloc/weights are host-prepped per slot).  PE matmuls
  lhsT=rows (fp32r, a free bitcast of the gathered fp32) x rhs=mask
  accumulate yT[d, token] into PSUM; the spill slabs use a 512-wide mask
  over the whole half and accumulate last.  Stage 2 (out_proj) contracts
  yT with w_out.T (fp32r) per 128-token group, adds bias on DVE, DMAs out.

  The dma_gather Q7 ucode reads index i of a call from the idx tile at
  [16 + i%16, i//16] on the NEFF path (queue 0 channel base), while the
  bass-level interpreter reads [i%16, i//16]; the host writes both bands.
"""

import numpy as np

B, T, K = 4, 2048, 16
C, D, E = 262144, 256, 512
NCORES = 8
NTOK = B * T                      # 8192 tokens
TPC = NTOK // NCORES              # 1024 tokens per core
P = 128
HALVES = 2
HTOK = TPC // HALVES              # 512 tokens per half
NB = 8                            # value chunks
CHUNK = C // NB                   # 32768 rows, int16-addressable
NW = 8                            # subgroups per half
WTOK = HTOK // NW                 # 64 tokens per subgroup
NIDX_CALL = NW * P                # 1024 indices per dma_gather call
SPILL_SLABS = 4                   # per half
SPILL_CAP = SPILL_SLABS * P       # 512
MAIN_SLABS = HALVES * NB * NW     # 128
SPILL_TOT = HALVES * SPILL_SLABS  # 8

_CACHE = {}


def _build_bass():
    import concourse.bass as bass
    import concourse.mybir as mybir
    from concourse import bacc, library_config
    from concourse.tile import TileContext

    fp32 = mybir.dt.float32
    f32r = mybir.dt.float32r
    i16 = mybir.dt.int16
    i32 = mybir.dt.int32
    eq = mybir.AluOpType.is_equal
    mul = mybir.AluOpType.mult
    nc = bacc.Bacc(
        "TRN2", target_bir_lowering=False, debug=False, num_devices=NCORES
    )

    kb = nc.dram_tensor("kb", [C, D], f32r, kind="ExternalInput")
    idx16 = nc.dram_tensor("idx16", [P, HALVES * NB * (NIDX_CALL // 16)], i16,
                           kind="ExternalInput")
    idxsp = nc.dram_tensor("idxsp", [P, SPILL_TOT], i32, kind="ExternalInput")
    wslot = nc.dram_tensor("wslot", [P, MAIN_SLABS], fp32, kind="ExternalInput")
    tokloc = nc.dram_tensor("tokloc", [P, MAIN_SLABS], i16, kind="ExternalInput")
    wsp = nc.dram_tensor("wsp", [P, SPILL_TOT], fp32, kind="ExternalInput")
    toksp = nc.dram_tensor("toksp", [P, SPILL_TOT], i16, kind="ExternalInput")
    iota64 = nc.dram_tensor("iota64", [P, WTOK], i16, kind="ExternalInput")
    iota512 = nc.dram_tensor("iota512", [P, HTOK], i16, kind="ExternalInput")
    wout = nc.dram_tensor("wout", [P, 2 * E], f32r, kind="ExternalInput")
    bias = nc.dram_tensor("bias", [P, E], fp32, kind="ExternalInput")
    out = nc.dram_tensor("out", [TPC, E], fp32, kind="ExternalOutput")

    COLS = NIDX_CALL // 16  # idx16 columns per call

    with TileContext(nc) as tc:
        with (
            tc.tile_pool(name="const", bufs=1) as cpool,
            tc.tile_pool(name="gath", bufs=10) as gpool,
            tc.tile_pool(name="mask", bufs=4) as mpool,
            tc.tile_pool(name="spill", bufs=2) as sppool,
            tc.tile_pool(name="spmask", bufs=2) as smpool,
            tc.tile_pool(name="y", bufs=2) as ypool,
            tc.tile_pool(name="o", bufs=8) as opool,
            tc.tile_pool(name="psy", bufs=2, space="PSUM") as psy,
            tc.tile_pool(name="pso", bufs=2, space="PSUM") as pso,
        ):
            nc.gpsimd.load_library(library_config.mlp)

            idx_sb = cpool.tile([P, HALVES * NB * COLS], i16)
            nc.sync.dma_start(out=idx_sb[:], in_=idx16[:, :])
            idxsp_sb = cpool.tile([P, SPILL_TOT], i32)
            nc.sync.dma_start(out=idxsp_sb[:], in_=idxsp[:, :])
            w_sb = cpool.tile([P, MAIN_SLABS], fp32)
            nc.sync.dma_start(out=w_sb[:], in_=wslot[:, :])
            tl_sb = cpool.tile([P, MAIN_SLABS], i16)
            nc.sync.dma_start(out=tl_sb[:], in_=tokloc[:, :])
            wsp_sb = cpool.tile([P, SPILL_TOT], fp32)
            nc.sync.dma_start(out=wsp_sb[:], in_=wsp[:, :])
            tsp_sb = cpool.tile([P, SPILL_TOT], i16)
            nc.sync.dma_start(out=tsp_sb[:], in_=toksp[:, :])
            io64_sb = cpool.tile([P, WTOK], i16)
            nc.sync.dma_start(out=io64_sb[:], in_=iota64[:, :])
            io512_sb = cpool.tile([P, HTOK], i16)
            nc.sync.dma_start(out=io512_sb[:], in_=iota512[:, :])
            wo_sb = cpool.tile([P, 2 * E], f32r)
            nc.sync.dma_start(out=wo_sb[:], in_=wout[:, :])
            b_sb = cpool.tile([P, E], fp32)
            nc.sync.dma_start(out=b_sb[:], in_=bias[:, :])

            for h in range(HALVES):
                yt = psy.tile([P, 2 * HTOK], fp32, tag="yt")
                # start=True zeroes the whole 2KB psum zero-region, which
                # would wipe earlier 64-col writes in the same bank: zero the
                # banks once and accumulate-only (start=False everywhere).
                nc.vector.memset(yt[:], 0.0)

                gs = []
                for b in range(NB):
                    g = gpool.tile([P, NW, D], f32r, tag="g")
                    col0 = (h * NB + b) * COLS
                    nc.gpsimd.dma_gather(
                        out_ap=g[:],
                        in_ap=kb[b * CHUNK:(b + 1) * CHUNK, :],
                        idxs_ap=idx_sb[:, col0:col0 + COLS],
                        num_idxs=NIDX_CALL,
                        num_idxs_reg=NIDX_CALL,
                        elem_size=D,
                    )
                    gs.append(g)

                if h == 0:
                    # issue BOTH halves' spill gathers now: their desc-gen
                    # overlaps h0's transfers and the data arrives well before
                    # each half's epilogue (instead of queueing after all
                    # gathers and serializing the tail).
                    sp_tiles = []
                    for hh in range(HALVES):
                        sp = sppool.tile([P, SPILL_SLABS, D], f32r, tag="sp")
                        for s in range(SPILL_SLABS):
                            col = hh * SPILL_SLABS + s
                            nc.gpsimd.indirect_dma_start(
                                out=sp[:, s, :],
                                out_offset=None,
                                in_=kb[:, :],
                                in_offset=bass.IndirectOffsetOnAxis(
                                    ap=idxsp_sb[:, col:col + 1], axis=0
                                ),
                            )
                        sp_tiles.append(sp)
                sp = sp_tiles[h]

                # mask-matmul reduction, bucket by bucket
                for b in range(NB):
                    blk = (h * NB + b) * NW
                    mask = mpool.tile([P, NW, WTOK], f32r, tag="m")
                    nc.vector.tensor_tensor(
                        out=mask[:],
                        in0=io64_sb[:].unsqueeze(1).broadcast_to([P, NW, WTOK]),
                        in1=tl_sb[:, blk:blk + NW].unsqueeze(2)
                            .broadcast_to([P, NW, WTOK]),
                        op=eq,
                    )
                    nc.vector.tensor_tensor(
                        out=mask[:],
                        in0=mask[:],
                        in1=w_sb[:, blk:blk + NW].unsqueeze(2)
                            .broadcast_to([P, NW, WTOK]),
                        op=mul,
                    )
                    for j in range(NW):
                        for ch in range(2):
                            nc.tensor.matmul(
                                out=yt[:, ch * HTOK + j * WTOK:
                                       ch * HTOK + (j + 1) * WTOK],
                                lhsT=gs[b][:, j, ch * P:(ch + 1) * P],
                                rhs=mask[:, j, :],
                                start=False,
                                stop=False,
                                skip_group_check=True,
                            )

                msp = smpool.tile([P, SPILL_SLABS, HTOK], f32r, tag="msp")
                sblk = h * SPILL_SLABS
                nc.vector.tensor_tensor(
                    out=msp[:],
                    in0=io512_sb[:].unsqueeze(1)
                        .broadcast_to([P, SPILL_SLABS, HTOK]),
                    in1=tsp_sb[:, sblk:sblk + SPILL_SLABS].unsqueeze(2)
                        .broadcast_to([P, SPILL_SLABS, HTOK]),
                    op=eq,
                )
                nc.vector.tensor_tensor(
                    out=msp[:],
                    in0=msp[:],
                    in1=wsp_sb[:, sblk:sblk + SPILL_SLABS].unsqueeze(2)
                        .broadcast_to([P, SPILL_SLABS, HTOK]),
                    op=mul,
                )
                for s in range(SPILL_SLABS):
                    for ch in range(2):
                        nc.tensor.matmul(
                            out=yt[:, ch * HTOK:(ch + 1) * HTOK],
                            lhsT=sp[:, s, ch * P:(ch + 1) * P],
                            rhs=msp[:, s, :],
                            start=False,
                            stop=(s == SPILL_SLABS - 1),
                            skip_group_check=True,
                        )

                yb = ypool.tile([P, 2 * HTOK], f32r, tag="yb")
                for g4 in range(HTOK // P):
                    for ch in range(2):
                        nc.vector.tensor_copy(
                            out=yb[:, ch * HTOK + g4 * P:ch * HTOK + (g4 + 1) * P],
                            in_=yt[:, ch * HTOK + g4 * P:ch * HTOK + (g4 + 1) * P],
                        )

                for g4 in range(HTOK // P):
                    ops = pso.tile([P, E], fp32, tag="ops")
                    for ch in range(2):
                        nc.tensor.matmul(
                            out=ops[:],
                            lhsT=yb[:, ch * HTOK + g4 * P:
                                    ch * HTOK + (g4 + 1) * P],
                            rhs=wo_sb[:, ch * E:(ch + 1) * E],
                            start=(ch == 0),
                            stop=(ch == 1),
                        )
                    osb = opool.tile([P, E], fp32, tag="osb")
                    nc.vector.tensor_add(out=osb[:], in0=ops[:], in1=b_sb[:])
                    row0 = (h * (HTOK // P) + g4) * P
                    nc.sync.dma_start(out=out[row0:row0 + P, :], in_=osb[:])

    nc.compile()
    return nc


def _host_prep(weights, indexes, w_out, b_out):
    """Bucket/sort (token,k) pairs per core and build all device-side arrays."""
    wflat = np.ascontiguousarray(weights, dtype=np.float32).reshape(NTOK, K)
    iflat = np.ascontiguousarray(indexes).reshape(NTOK, K).astype(np.int64)

    woutT = np.ascontiguousarray(w_out, dtype=np.float32).T      # [D, E]
    wout_host = np.ascontiguousarray(
        woutT.reshape(2, P, E).transpose(1, 0, 2).reshape(P, 2 * E)
    )
    bias_host = np.ascontiguousarray(
        np.broadcast_to(np.asarray(b_out, dtype=np.float32), (P, E))
    )
    iota64_h = np.ascontiguousarray(
        np.broadcast_to(np.arange(WTOK, dtype=np.int16), (P, WTOK))
    )
    iota512_h = np.ascontiguousarray(
        np.broadcast_to(np.arange(HTOK, dtype=np.int16), (P, HTOK))
    )

    COLS = NIDX_CALL // 16
    in_maps = []
    for c in range(NCORES):
        ic = iflat[c * TPC:(c + 1) * TPC].ravel()          # [16384]
        wc = wflat[c * TPC:(c + 1) * TPC].ravel()
        t = np.repeat(np.arange(TPC, dtype=np.int64), K)   # token per pair

        h = t // HTOK
        wsub = (t % HTOK) // WTOK
        b = ic // CHUNK
        key = (h * NB + b) * NW + wsub                     # 0..127 slab id

        order = np.argsort(key, kind="stable")
        ks = key[order]
        iv = ic[order]
        wv = wc[order]
        tv = t[order]
        starts = np.searchsorted(ks, np.arange(MAIN_SLABS))
        rank = np.arange(TPC * K) - starts[ks]

        idx16_host = np.zeros((P, HALVES * NB * COLS), np.int16)
        wslot_host = np.zeros((P, MAIN_SLABS), np.float32)
        tokloc_host = np.zeros((P, MAIN_SLABS), np.int16)
        idxsp_host = np.zeros((P, SPILL_TOT), np.int32)
        wsp_host = np.zeros((P, SPILL_TOT), np.float32)
        toksp_host = np.zeros((P, SPILL_TOT), np.int16)

        main = rank < P
        mk, mr = ks[main], rank[main]
        mi, mw, mt = iv[main], wv[main], tv[main]
        mh = mk // (NB * NW)
        mb = (mk // NW) % NB
        mj = mk % NW
        slot = mj * P + mr                                 # slot within call
        col = (mh * NB + mb) * COLS + slot // 16
        idx_local = (mi - mb * CHUNK).astype(np.int16)
        idx16_host[slot % 16, col] = idx_local             # interp layout
        idx16_host[16 + slot % 16, col] = idx_local        # NEFF Q7 layout
        wslot_host[mr, mk] = mw
        tokloc_host[mr, mk] = (mt - (mh * HTOK + mj * WTOK)).astype(np.int16)

        sh = ks[~main] // (NB * NW)                        # spill half
        si, sw, st = iv[~main], wv[~main], tv[~main]
        for hh in range(HALVES):
            sel = sh == hh
            n = int(sel.sum())
            if n > SPILL_CAP:
                raise ValueError(
                    f"spill overflow: core {c} half {hh} needs {n} > {SPILL_CAP}"
                )
            r = np.arange(n)
            idxsp_host[r % P, hh * SPILL_SLABS + r // P] = si[sel]
            wsp_host[r % P, hh * SPILL_SLABS + r // P] = sw[sel]
            toksp_host[r % P, hh * SPILL_SLABS + r // P] = (
                st[sel] - hh * HTOK
            ).astype(np.int16)

        in_maps.append({
            "idx16": idx16_host,
            "idxsp": idxsp_host,
            "wslot": wslot_host,
            "tokloc": tokloc_host,
            "wsp": wsp_host,
            "toksp": toksp_host,
            "iota64": iota64_h,
            "iota512": iota512_h,
            "wout": wout_host,
            "bias": bias_host,
        })
    return in_maps


def kernel(weights, indexes, knowledge_base, w_out, b_out):
    from concourse.bass_utils import run_bass_kernel_spmd

    if "nc" not in _CACHE:
        _CACHE["nc"] = _build_bass()
    nc = _CACHE["nc"]

    kb_host = np.ascontiguousarray(knowledge_base, dtype=np.float32)
    in_maps = _host_prep(weights, indexes, w_out, b_out)
    for m in in_maps:
        m["kb"] = kb_host

    res = run_bass_kernel_spmd(nc, in_maps, list(range(NCORES)))
    out = np.concatenate([res.results[c]["out"] for c in range(NCORES)], axis=0)
    return out.reshape(B, T, E).astype(np.float32)
